# revision 15
# baseline (speedup 1.0000x reference)
"""Trainium2 Bass kernel for nn_DFBBlock (dense CNN block).

Strategy: data-parallel over batch B=8 across 8 NeuronCores (1 image each).
The power iteration for lip2 (batch-size-1, data-dependent while loop) runs
on the host in fp32 numpy replicating the reference's jax.lax.while_loop
semantics; the resulting scalars (tau, sigma) are folded into the conv
weights baked into the device program.

Device kernel per core (image 256x256, F=64 channels):
  z    = conv_t(u, tau*w)                       (64ch -> 1ch, 3x3, pad 1)
  out0 = clip(x - tau*(x*HtH) - z + tau*bias, 0, 1)
  q    = 2*out0 - x
  out1 = clip(conv(q, sigma*w) + u, -lambd, lambd)   (1ch -> 64ch)

Convs run on the TensorEngine as accumulating matmuls in fp16 (hi/lo split
operands available via TERMS_* for extra precision).
  conv_t: K=64 (channels), M=1, one matmul per 3x3 tap per 2-row chunk,
          shifts expressed in the rhs access pattern; 8-way packed via
          (row-group x col-group) tile positions.
  conv:   K=3 (dy taps from 3 row-shifted q copies), M=64, one matmul per
          dx tap; +u is folded in as identity-matrix matmuls into the same
          PSUM accumulation, so the epilogue is a single fused clip.
"""

import os
import numpy as np
from contextlib import ExitStack

import concourse.bacc as bacc
import concourse.mybir as mybir
import concourse.tile as tile
from concourse import bass_utils

F32 = mybir.dt.float32
F16 = mybir.dt.float16
OP = mybir.AluOpType

B, C, F, H, W, K = 8, 1, 64, 256, 256, 3
R = 64            # band rows
HB = R // 2       # half-band rows
NBANDS = H // R

# compensated-product terms per conv, as (data_lo?, weight_lo?) products.
# [(0,0)]               plain fp16            ~6.5e-4 end-to-end max abs err
# [(0,0),(0,1)]         + weight-lo term      ~3.9e-4
# [(0,0),(1,0),(0,1)]   full 3-term           ~2e-6 (conv part)
# Chosen at kernel() time from the actual weight scale (see kernel()).

_TRACE = [False]
_LAST_RESULTS = [None]
_LAST_PROGRAM = [None]


# ---------------------------------------------------------------- host math

def _f16_split(a):
    hi = a.astype(np.float16)
    lo = (a - hi.astype(np.float32)).astype(np.float16)
    return hi, lo


def _power_iteration_lip2(w):
    """Replicates reference._op_norm2 in fp32 numpy: power iteration for
    ||conv_t(conv(.))|| from a normalized ones image, max 300 iters,
    rel tol 1e-4 on the norm estimate."""
    w = np.asarray(w, np.float32).reshape(F, 3, 3)
    w9 = w.reshape(F, 9)

    def conv(x):                              # [H,W] -> [H*W, F]
        xp = np.zeros((H + 2, W + 2), np.float32)
        xp[1:-1, 1:-1] = x
        cols = np.empty((H * W, 9), np.float32)
        for a in range(3):
            for b in range(3):
                cols[:, a * 3 + b] = xp[a:a + H, b:b + W].reshape(-1)
        return cols @ w9.T

    def conv_t(y):                            # [H*W, F] -> [H,W]
        s = (y @ w9).reshape(H, W, 9)
        z = np.zeros((H + 2, W + 2), np.float32)
        for a in range(3):
            for b in range(3):
                # s tap (a,b) at (r',c') contributes to z[r'+a-1, c'+b-1]
                z[a:a + H, b:b + W] += s[:, :, a * 3 + b]
        return z[1:-1, 1:-1]

    x = np.full((H, W), 1.0, np.float32)
    x /= np.float32(np.linalg.norm(x.reshape(-1)))
    val = np.float32(1.0)
    tol = np.float32(1e-4)
    for _ in range(300):
        x2 = conv_t(conv(x))
        v2 = np.float32(np.linalg.norm(x2.reshape(-1)))
        rel = np.float32(abs(v2 - val) / val)
        x = x2 / v2
        val = v2
        if rel < tol:
            break
    return float(val)


# ------------------------------------------------------------ device kernel

def _build_program(tau, sigma, lambd_, ct_terms, cq_terms, debug=False):
    CT_HAS_WLO = any(t[1] for t in ct_terms)
    CQ_HAS_QLO = any(t[0] for t in cq_terms)
    CQ_HAS_WLO = any(t[1] for t in cq_terms)
    nc = bacc.Bacc("TRN2", target_bir_lowering=False, debug=False)

    uh_d = nc.dram_tensor("uh", [F, H * W], F16, kind="ExternalInput")
    ul_d = nc.dram_tensor("ul", [F, H * W], F16, kind="ExternalInput")
    x_d = nc.dram_tensor("x", [128, 512], F32, kind="ExternalInput")
    bias_d = nc.dram_tensor("bias", [128, 512], F32, kind="ExternalInput")
    diag_d = nc.dram_tensor("diag", [128, 512], F32, kind="ExternalInput")
    # conv_t weights (tau*w) [64, 9] fp16 (hi, and lo if TERMS_CT >= 3)
    wzh_d = nc.dram_tensor("wzh", [F, 9], F16, kind="ExternalInput")
    wzl_d = (nc.dram_tensor("wzl", [F, 9], F16, kind="ExternalInput")
             if CT_HAS_WLO else None)
    # conv(q) weights (sigma*w) [3, 3*F]: row i holds [j, f] -> w[f,0,i,j]
    wq_d = nc.dram_tensor("wq", [3, 3 * F], F16, kind="ExternalInput")
    wql_d = (nc.dram_tensor("wql", [3, 3 * F], F16, kind="ExternalInput")
             if CQ_HAS_WLO else None)
    eye_d = nc.dram_tensor("eye", [F, F], F16, kind="ExternalInput")

    out0_d = nc.dram_tensor("out0", [128, 512], F32, kind="ExternalOutput")
    out1_d = nc.dram_tensor("out1", [F, H * W], F32, kind="ExternalOutput")
    if debug:
        dbgz_d = nc.dram_tensor("dbg_z", [128, 512], F32, kind="ExternalOutput")
        dbgq_d = nc.dram_tensor("dbg_q", [128, 512], F32, kind="ExternalOutput")

    uh_v = uh_d.ap().rearrange("f (r c) -> f r c", c=W)
    ul_v = ul_d.ap().rearrange("f (r c) -> f r c", c=W)

    with tile.TileContext(nc) as tc, ExitStack() as ctx:
        const_pool = ctx.enter_context(tc.tile_pool(name="const", bufs=1))
        upool = ctx.enter_context(tc.tile_pool(name="u", bufs=3))
        plane_pool = ctx.enter_context(tc.tile_pool(name="plane", bufs=1))
        q3pool = ctx.enter_context(tc.tile_pool(name="q3", bufs=2))
        stpool = ctx.enter_context(tc.tile_pool(name="st", bufs=4))
        o1pool = ctx.enter_context(tc.tile_pool(name="o1", bufs=4))
        psA = ctx.enter_context(tc.tile_pool(name="psA", bufs=2, space="PSUM"))
        psQ = ctx.enter_context(tc.tile_pool(name="psQ", bufs=3, space="PSUM"))

        # ---- constants (replicated at partition bases 0 and 64)
        wzh = const_pool.tile([128, 9], F16)
        nc.sync.dma_start(wzh[0:F, :], wzh_d.ap())
        nc.sync.dma_start(wzh[64:64 + F, :], wzh_d.ap())
        if CT_HAS_WLO:
            wzl = const_pool.tile([128, 9], F16)
            nc.sync.dma_start(wzl[0:F, :], wzl_d.ap())
            nc.sync.dma_start(wzl[64:64 + F, :], wzl_d.ap())
        wq3 = const_pool.tile([128, 3, F], F16)
        nc.sync.dma_start(wq3[0:3, :, :],
                          wq_d.ap().rearrange("i (j f) -> i j f", f=F))
        nc.sync.dma_start(wq3[64:67, :, :],
                          wq_d.ap().rearrange("i (j f) -> i j f", f=F))
        if CQ_HAS_WLO:
            wq3l = const_pool.tile([128, 3, F], F16)
            nc.sync.dma_start(wq3l[0:3, :, :],
                              wql_d.ap().rearrange("i (j f) -> i j f", f=F))
            nc.sync.dma_start(wq3l[64:67, :, :],
                              wql_d.ap().rearrange("i (j f) -> i j f", f=F))
        eye = const_pool.tile([128, F], F16)
        nc.sync.dma_start(eye[0:F, :], eye_d.ap())
        nc.sync.dma_start(eye[64:64 + F, :], eye_d.ap())
        zeros_f16 = const_pool.tile([F, W], F16)
        nc.vector.memset(zeros_f16[:], 0.0)

        # ---- fp32 planes [128, 512]: partition p = image rows 2p, 2p+1
        x_sb = plane_pool.tile([128, 512], F32)
        bias_sb = plane_pool.tile([128, 512], F32)
        diag_sb = plane_pool.tile([128, 512], F32)
        z_sb = plane_pool.tile([128, 512], F32)
        t_sb = plane_pool.tile([128, 512], F32)
        o0_sb = plane_pool.tile([128, 512], F32)
        q_sb = plane_pool.tile([128, 512], F32)
        qh_sb = plane_pool.tile([128, 512], F16)
        ql_sb = (plane_pool.tile([128, 512], F16, name="ql_sb")
                 if CQ_HAS_QLO else None)
        nc.sync.dma_start(x_sb[:], x_d.ap())
        nc.sync.dma_start(bias_sb[:], bias_d.ap())
        nc.sync.dma_start(diag_sb[:], diag_d.ap())


        uts = {}

        def load_u(k):
            r0 = k * R
            uth = upool.tile([128, HB + 2, W], F16, name=f"uth{k}", tag="uth")
            utl = upool.tile([128, HB + 2, W], F16, name=f"utl{k}", tag="utl")
            for half in range(2):
                base = 64 * half
                lo = r0 + half * HB - 1
                hi = lo + HB + 2
                for ut, uv in ((uth, uh_v), (utl, ul_v)):
                    l, h_, d = lo, hi, 0
                    if l < 0:
                        nc.sync.dma_start(ut[base:base + F, 0, :], zeros_f16[:])
                        l, d = 0, 1
                    if h_ > H:
                        nc.sync.dma_start(ut[base:base + F, HB + 1, :],
                                          zeros_f16[:])
                        h_ = H
                    nc.sync.dma_start(ut[base:base + F, d:d + h_ - l, :],
                                      uv[:, l:h_, :])
            uts[k] = (uth, utl)

        # tap col windows: index by "out col reads src col + (idx-1)"
        # idx 0: out [1,256) <- src [0,255); idx 1: full; idx 2: out [0,255) <- src [1,256)
        WIN = [(1, 0, 255), (0, 0, 256), (0, 1, 255)]

        def conv_t_band(k):
            uth, utl = uts[k]
            for g in range(HB // 8):        # 4 groups of 4 chunks per half
                pss = [psA.tile([128, 512], F32, name=f"psA{k}_{g}_{i}",
                                tag=f"psA{i}") for i in range(2)]
                nmm = len(ct_terms) * 9
                imm = 0
                for (ulo, wlo) in ct_terms:
                    ut = utl if ulo else uth
                    wt = wzl if wlo else wzh
                    for b in [1, 0, 2]:
                        # out col c reads u col c+1-b -> window idx (2-b)
                        ob, sb_, width = WIN[2 - b]
                        for a in range(3):
                            tap = a * 3 + b
                            imm += 1
                            for half in range(2):
                                base = 64 * half
                                for cc in range(4):
                                    y = (g * 4 + cc) * 2
                                    col = 32 * cc
                                    rhs = ut[base:base + F,
                                             y + 2 - a:y + 4 - a,
                                             sb_:sb_ + width]
                                    out = pss[half][col:col + 1, :].rearrange(
                                        "p (r c) -> p r c", r=2)[:, :, ob:ob + width]
                                    nc.tensor.matmul(
                                        out, wt[base:base + F, tap:tap + 1], rhs,
                                        start=(imm == 1), stop=(imm == nmm),
                                        tile_position=(base, col),
                                        skip_group_check=True,
                                    )
                for half in range(2):
                    st = stpool.tile([128, 512], F32, name=f"st{k}_{g}_{half}",
                                     tag="st")
                    # psum -> sbuf staging on the scalar engine (ACT is idle)
                    if half == 0:
                        nc.scalar.copy(st[:], pss[half][:])
                    else:
                        nc.vector.tensor_copy(st[:], pss[half][:])
                    gbase = k * 32 + half * 16 + g * 4
                    nc.sync.dma_start(z_sb[gbase:gbase + 4, :], st[0:97:32, :])

        def dx_band(k):
            sl = slice(32 * k, 32 * k + 32)
            x_ = x_sb[sl, :]
            # elementwise chain on gpsimd (SBUF only), conversions on DVE
            nc.gpsimd.tensor_mul(t_sb[sl, :], x_, diag_sb[sl, :])
            nc.vector.scalar_tensor_tensor(
                t_sb[sl, :], t_sb[sl, :], -tau, x_, OP.mult, OP.add)
            nc.gpsimd.tensor_sub(t_sb[sl, :], t_sb[sl, :], z_sb[sl, :])
            nc.vector.scalar_tensor_tensor(
                t_sb[sl, :], bias_sb[sl, :], tau, t_sb[sl, :], OP.mult, OP.add)
            nc.gpsimd.tensor_scalar(
                o0_sb[sl, :], t_sb[sl, :], 0.0, 1.0, OP.max, OP.min)
            # q = 2*out0 - x
            nc.vector.scalar_tensor_tensor(
                q_sb[sl, :], o0_sb[sl, :], 2.0, x_, OP.mult, OP.subtract)
            nc.vector.tensor_copy(qh_sb[sl, :], q_sb[sl, :])
            if CQ_HAS_QLO:
                nc.vector.tensor_sub(t_sb[sl, :], q_sb[sl, :], qh_sb[sl, :])
                nc.vector.tensor_copy(ql_sb[sl, :], t_sb[sl, :])

        def build_q3(k, qplane, q3t):
            """q3t partition (base + i) tile row t = q row (hs - 2 + i + t),
            zeros outside the image."""
            r0 = k * R
            for half in range(2):
                base = 64 * half
                hs = r0 + half * HB
                for i in range(3):
                    p = base + i
                    rlo = hs - 2 + i
                    rhi = rlo + HB + 2
                    dlo = 0
                    if rlo < 0:
                        for d in range(-rlo):
                            nc.sync.dma_start(q3t[p:p + 1, d, :],
                                              zeros_f16[0:1, :])
                        dlo, rlo = -rlo, 0
                    if rhi > H:
                        for d in range(rhi - H):
                            nc.sync.dma_start(
                                q3t[p:p + 1, HB + 2 - (rhi - H) + d, :],
                                zeros_f16[0:1, :])
                        rhi = H
                    nrows = rhi - rlo
                    plo = rlo // 2
                    if rlo % 2 == 1:
                        nc.sync.dma_start(q3t[p:p + 1, dlo, :],
                                          qplane[plo:plo + 1, 256:512])
                        plo += 1
                        dlo += 1
                        nrows -= 1
                    neven = nrows // 2
                    if neven:
                        nc.sync.dma_start(
                            q3t[p:p + 1, dlo:dlo + 2 * neven, :],
                            qplane[plo:plo + neven, :])
                    if nrows % 2 == 1:
                        nc.sync.dma_start(
                            q3t[p:p + 1, dlo + 2 * neven, :],
                            qplane[plo + neven:plo + neven + 1, 0:256])

        def conv_q_band(k):
            q3h = q3pool.tile([128, HB + 2, W], F16, name=f"q3h{k}", tag="q3h")
            build_q3(k, qh_sb, q3h)
            q3l = None
            if CQ_HAS_QLO:
                q3l = q3pool.tile([128, HB + 2, W], F16, name=f"q3l{k}",
                                  tag="q3l")
                build_q3(k, ql_sb, q3l)
            uth, utl = uts[k]
            r0 = k * R
            for pair in range(HB // 2):     # 16 chunk-pairs per band
                ps = psQ.tile([128, 512], F32, name=f"psQ{k}_{pair}", tag="psQ")
                y = pair * 2
                for half in range(2):
                    base = 64 * half
                    # alternate col sets by pair parity so two consecutive
                    # pairs' matmul streams use disjoint (row, col) groups
                    # and run concurrently on the PE array
                    cb = 64 * (half ^ (pair & 1))
                    # u identity-accumulate (exact fp16 hi+lo pass-through)
                    nc.tensor.matmul(
                        ps[cb:cb + F, :], eye[base:base + F, :],
                        uth[base:base + F, y + 1:y + 3, :],
                        start=True, stop=False, tile_position=(base, cb),
                        skip_group_check=True)
                    nc.tensor.matmul(
                        ps[cb:cb + F, :], eye[base:base + F, :],
                        utl[base:base + F, y + 1:y + 3, :],
                        start=False, stop=False, tile_position=(base, cb),
                        skip_group_check=True)
                    nmm = 3 * len(cq_terms)
                    imm = 0
                    for (qlo, wlo) in cq_terms:
                        q3 = q3l if qlo else q3h
                        wt = wq3l if wlo else wq3
                        for j in [1, 0, 2]:
                            ob, sb_, width = WIN[j]
                            imm += 1
                            rhs = q3[base:base + 3, y + 1:y + 3,
                                     sb_:sb_ + width]
                            out = ps[cb:cb + F, :].rearrange(
                                "p (r c) -> p r c", r=2)[:, :, ob:ob + width]
                            nc.tensor.matmul(
                                out, wt[base:base + 3, j, :], rhs,
                                start=False, stop=(imm == nmm),
                                tile_position=(base, cb),
                                skip_group_check=True)
                # fused epilogue: out1 = clip(psum, -lambd, lambd)
                o1 = o1pool.tile([128, 512], F32, name=f"o1_{k}_{pair}",
                                 tag="o1")
                nc.vector.tensor_scalar(
                    o1[:], ps[:], lambd_, -lambd_, OP.min, OP.max)
                # single DMA: partitions (colset, ch) -> rows; on odd pairs
                # the halves sit swapped in psum partitions
                dst = out1_d.ap().rearrange(
                    "f (bl h r c) -> f bl h r c",
                    bl=NBANDS, h=2, c=W)[:, k, :, y:y + 2, :]
                dst = dst.transpose([1, 0, 2, 3])
                if pair & 1:
                    nc.sync.dma_start(dst[1], o1[0:64, :])
                    nc.sync.dma_start(dst[0], o1[64:128, :])
                else:
                    nc.sync.dma_start(dst, o1[:])

        # ---------------- schedule
        for k in range(NBANDS):
            load_u(k)
            conv_t_band(k)
            dx_band(k)
            if k >= 1:
                conv_q_band(k - 1)
        conv_q_band(NBANDS - 1)
        nc.sync.dma_start(out0_d.ap(), o0_sb[:])
        if debug:
            nc.sync.dma_start(dbgz_d.ap(), z_sb[:])
            nc.sync.dma_start(dbgq_d.ap(), q_sb[:])

    nc.compile()
    return nc


# ---------------------------------------------------------------- wrapper

def kernel(x_in, u_in, bias, beta, lambd, HtH_diag, w, log_sigma):
    x_in = np.asarray(x_in, np.float32)
    u_in = np.asarray(u_in, np.float32)
    bias = np.asarray(bias, np.float32)
    HtH_diag = np.asarray(HtH_diag, np.float32)
    w = np.asarray(w, np.float32)
    beta_f = float(np.asarray(beta).reshape(-1)[0])
    lambd_f = float(np.asarray(lambd).reshape(-1)[0])
    log_sigma_f = float(np.asarray(log_sigma).reshape(-1)[0])

    lip2 = _power_iteration_lip2(w)
    sigma = float(np.exp(np.float32(log_sigma_f)) + np.float32(0.05))
    tau = float(np.float32(0.99) / np.float32(beta_f / 2 + sigma * lip2))

    # pick precision terms from the actual quantization-error scale:
    # conv_q error ~ 3 taps * |sigma*w| * 2^-11 * |q|; add compensation
    # terms when the plain-fp16 estimate threatens ~5e-4 absolute.
    w_std = float(np.std(np.float32(sigma) * w))
    cq_err_est = 3.0 * w_std * 2.0 ** -11 * 3.0
    ct_terms = [(0, 0), (0, 1)]
    if cq_err_est > 6e-4:
        cq_terms = [(0, 0), (1, 0), (0, 1)]
    else:
        cq_terms = [(0, 0), (0, 1)]

    nc = _build_program(
        tau, sigma, lambd_f, ct_terms, cq_terms,
        debug=bool(int(os.environ.get("KERNEL_DEBUG", "0"))))
    _LAST_PROGRAM[0] = nc

    w9 = w.reshape(F, 9)
    wzh, wzl = _f16_split(np.float32(tau) * w9)
    # wq[i, j*F + f] = sigma * w[f, 0, i, j]
    wq = np.ascontiguousarray(
        (np.float32(sigma) * w.reshape(F, 3, 3)).transpose(1, 2, 0)
    ).reshape(3, 3 * F)
    wqh, wql = _f16_split(wq)
    eye = np.eye(F, dtype=np.float16)
    diag_plane = np.ascontiguousarray(HtH_diag.reshape(128, 512))

    in_maps = []
    for b in range(B):
        uh, ul = _f16_split(u_in[b].reshape(F, H * W))
        m = {
            "uh": uh, "ul": ul,
            "x": np.ascontiguousarray(x_in[b, 0].reshape(128, 512)),
            "bias": np.ascontiguousarray(bias[b].reshape(128, 512)),
            "diag": diag_plane,
            "wzh": wzh, "wq": wqh, "eye": eye,
        }
        if any(t[1] for t in ct_terms):
            m["wzl"] = wzl
        if any(t[1] for t in cq_terms):
            m["wql"] = wql
        in_maps.append(m)

    res = bass_utils.run_bass_kernel_spmd(
        nc, in_maps, core_ids=list(range(B)), trace=_TRACE[0])
    _LAST_RESULTS[0] = res

    out0 = np.stack([res.results[b]["out0"].reshape(1, H, W)
                     for b in range(B)])
    out1 = np.stack([res.results[b]["out1"].reshape(F, H, W)
                     for b in range(B)])
    return out0, out1


# revision 16
# speedup vs baseline: 1.4513x; 1.4513x over previous
"""Trainium2 Bass kernel for nn_DFBBlock (dense CNN block).

Strategy: data-parallel over batch B=8 across 8 NeuronCores (1 image each).
The power iteration for lip2 (batch-size-1, data-dependent while loop) runs
on the host in fp32 numpy replicating the reference's jax.lax.while_loop
semantics; the resulting scalars (tau, sigma) are folded into the conv
weights baked into the device program.

Device kernel per core (image 256x256, F=64 channels):
  z    = conv_t(u, tau*w)                       (64ch -> 1ch, 3x3, pad 1)
  out0 = clip(x - tau*(x*HtH) - z + tau*bias, 0, 1)
  q    = 2*out0 - x
  out1 = clip(conv(q, sigma*w) + u, -lambd, lambd)   (1ch -> 64ch)

Convs run on the TensorEngine as accumulating matmuls in fp16 (hi/lo split
operands available via TERMS_* for extra precision).
  conv_t: K=64 (channels), M=1, one matmul per 3x3 tap per 2-row chunk,
          shifts expressed in the rhs access pattern; 8-way packed via
          (row-group x col-group) tile positions.
  conv:   K=3 (dy taps from 3 row-shifted q copies), M=64, one matmul per
          dx tap; +u is folded in as identity-matrix matmuls into the same
          PSUM accumulation, so the epilogue is a single fused clip.
"""

import os
import numpy as np
from contextlib import ExitStack

import concourse.bacc as bacc
import concourse.mybir as mybir
import concourse.tile as tile
from concourse import bass_utils

F32 = mybir.dt.float32
F16 = mybir.dt.float16
OP = mybir.AluOpType

B, C, F, H, W, K = 8, 1, 64, 256, 256, 3
R = 64            # band rows
HB = R // 2       # half-band rows
NBANDS = H // R

# compensated-product terms per conv, as (data_lo?, weight_lo?) products.
# [(0,0)]               plain fp16            ~6.5e-4 end-to-end max abs err
# [(0,0),(0,1)]         + weight-lo term      ~3.9e-4
# [(0,0),(1,0),(0,1)]   full 3-term           ~2e-6 (conv part)
# Chosen at kernel() time from the actual weight scale (see kernel()).

_TRACE = [False]
_LAST_RESULTS = [None]
_LAST_PROGRAM = [None]


# ---------------------------------------------------------------- host math

def _f16_split(a):
    hi = a.astype(np.float16)
    lo = (a - hi.astype(np.float32)).astype(np.float16)
    return hi, lo


def _power_iteration_lip2(w):
    """Replicates reference._op_norm2 in fp32 numpy: power iteration for
    ||conv_t(conv(.))|| from a normalized ones image, max 300 iters,
    rel tol 1e-4 on the norm estimate."""
    w = np.asarray(w, np.float32).reshape(F, 3, 3)
    w9 = w.reshape(F, 9)

    def conv(x):                              # [H,W] -> [H*W, F]
        xp = np.zeros((H + 2, W + 2), np.float32)
        xp[1:-1, 1:-1] = x
        cols = np.empty((H * W, 9), np.float32)
        for a in range(3):
            for b in range(3):
                cols[:, a * 3 + b] = xp[a:a + H, b:b + W].reshape(-1)
        return cols @ w9.T

    def conv_t(y):                            # [H*W, F] -> [H,W]
        s = (y @ w9).reshape(H, W, 9)
        z = np.zeros((H + 2, W + 2), np.float32)
        for a in range(3):
            for b in range(3):
                # s tap (a,b) at (r',c') contributes to z[r'+a-1, c'+b-1]
                z[a:a + H, b:b + W] += s[:, :, a * 3 + b]
        return z[1:-1, 1:-1]

    x = np.full((H, W), 1.0, np.float32)
    x /= np.float32(np.linalg.norm(x.reshape(-1)))
    val = np.float32(1.0)
    tol = np.float32(1e-4)
    for _ in range(300):
        x2 = conv_t(conv(x))
        v2 = np.float32(np.linalg.norm(x2.reshape(-1)))
        rel = np.float32(abs(v2 - val) / val)
        x = x2 / v2
        val = v2
        if rel < tol:
            break
    return float(val)


# ------------------------------------------------------------ device kernel

def _build_program(tau, sigma, lambd_, ct_terms, cq_terms, debug=False):
    CT_HAS_WLO = any(t[1] for t in ct_terms)
    CQ_HAS_QLO = any(t[0] for t in cq_terms)
    CQ_HAS_WLO = any(t[1] for t in cq_terms)
    nc = bacc.Bacc("TRN2", target_bir_lowering=False, debug=False)

    uh_d = nc.dram_tensor("uh", [F, H * W], F16, kind="ExternalInput")
    ul_d = nc.dram_tensor("ul", [F, H * W], F16, kind="ExternalInput")
    x_d = nc.dram_tensor("x", [128, 512], F32, kind="ExternalInput")
    bias_d = nc.dram_tensor("bias", [128, 512], F32, kind="ExternalInput")
    diag_d = nc.dram_tensor("diag", [128, 512], F32, kind="ExternalInput")
    # conv_t weights (tau*w) [64, 9] fp16 (hi, and lo if TERMS_CT >= 3)
    wzh_d = nc.dram_tensor("wzh", [F, 9], F16, kind="ExternalInput")
    wzl_d = (nc.dram_tensor("wzl", [F, 9], F16, kind="ExternalInput")
             if CT_HAS_WLO else None)
    # conv(q) weights (sigma*w) [3, 3*F]: row i holds [j, f] -> w[f,0,i,j]
    wq_d = nc.dram_tensor("wq", [3, 3 * F], F16, kind="ExternalInput")
    wql_d = (nc.dram_tensor("wql", [3, 3 * F], F16, kind="ExternalInput")
             if CQ_HAS_WLO else None)
    eye_d = nc.dram_tensor("eye", [F, F], F16, kind="ExternalInput")

    out0_d = nc.dram_tensor("out0", [128, 512], F32, kind="ExternalOutput")
    out1_d = nc.dram_tensor("out1", [F, H * W], F32, kind="ExternalOutput")
    if debug:
        dbgz_d = nc.dram_tensor("dbg_z", [128, 512], F32, kind="ExternalOutput")
        dbgq_d = nc.dram_tensor("dbg_q", [128, 512], F32, kind="ExternalOutput")

    uh_v = uh_d.ap().rearrange("f (r c) -> f r c", c=W)
    ul_v = ul_d.ap().rearrange("f (r c) -> f r c", c=W)

    with tile.TileContext(nc) as tc, ExitStack() as ctx:
        const_pool = ctx.enter_context(tc.tile_pool(name="const", bufs=1))
        upool = ctx.enter_context(tc.tile_pool(name="u", bufs=3))
        plane_pool = ctx.enter_context(tc.tile_pool(name="plane", bufs=1))
        q3pool = ctx.enter_context(tc.tile_pool(name="q3", bufs=2))
        stpool = ctx.enter_context(tc.tile_pool(name="st", bufs=4))
        o1pool = ctx.enter_context(tc.tile_pool(name="o1", bufs=4))
        psA = ctx.enter_context(tc.tile_pool(name="psA", bufs=2, space="PSUM"))
        psQ = ctx.enter_context(tc.tile_pool(name="psQ", bufs=4, space="PSUM"))

        # ---- constants (replicated at partition bases 0 and 64)
        wzh = const_pool.tile([128, 9], F16)
        nc.sync.dma_start(wzh[0:F, :], wzh_d.ap())
        nc.sync.dma_start(wzh[64:64 + F, :], wzh_d.ap())
        if CT_HAS_WLO:
            wzl = const_pool.tile([128, 9], F16)
            nc.sync.dma_start(wzl[0:F, :], wzl_d.ap())
            nc.sync.dma_start(wzl[64:64 + F, :], wzl_d.ap())
        wq3 = const_pool.tile([128, 3, F], F16)
        nc.sync.dma_start(wq3[0:3, :, :],
                          wq_d.ap().rearrange("i (j f) -> i j f", f=F))
        nc.sync.dma_start(wq3[64:67, :, :],
                          wq_d.ap().rearrange("i (j f) -> i j f", f=F))
        if CQ_HAS_WLO:
            wq3l = const_pool.tile([128, 3, F], F16)
            nc.sync.dma_start(wq3l[0:3, :, :],
                              wql_d.ap().rearrange("i (j f) -> i j f", f=F))
            nc.sync.dma_start(wq3l[64:67, :, :],
                              wql_d.ap().rearrange("i (j f) -> i j f", f=F))
        eye = const_pool.tile([128, F], F16)
        nc.sync.dma_start(eye[0:F, :], eye_d.ap())
        nc.sync.dma_start(eye[64:64 + F, :], eye_d.ap())
        zeros_f16 = const_pool.tile([F, W], F16)
        nc.vector.memset(zeros_f16[:], 0.0)

        # ---- fp32 planes [128, 512]: partition p = image rows 2p, 2p+1
        x_sb = plane_pool.tile([128, 512], F32)
        bias_sb = plane_pool.tile([128, 512], F32)
        diag_sb = plane_pool.tile([128, 512], F32)
        z_sb = plane_pool.tile([128, 512], F32)
        t_sb = plane_pool.tile([128, 512], F32)
        o0_sb = plane_pool.tile([128, 512], F32)
        q_sb = plane_pool.tile([128, 512], F32)
        qh_sb = plane_pool.tile([128, 512], F16)
        ql_sb = (plane_pool.tile([128, 512], F16, name="ql_sb")
                 if CQ_HAS_QLO else None)
        nc.sync.dma_start(x_sb[:], x_d.ap())
        nc.sync.dma_start(bias_sb[:], bias_d.ap())
        nc.sync.dma_start(diag_sb[:], diag_d.ap())


        uts = {}

        def load_u(k):
            r0 = k * R
            uth = upool.tile([128, HB + 2, W], F16, name=f"uth{k}", tag="uth")
            utl = upool.tile([128, HB + 2, W], F16, name=f"utl{k}", tag="utl")
            for half in range(2):
                base = 64 * half
                lo = r0 + half * HB - 1
                hi = lo + HB + 2
                for ut, uv in ((uth, uh_v), (utl, ul_v)):
                    l, h_, d = lo, hi, 0
                    if l < 0:
                        nc.sync.dma_start(ut[base:base + F, 0, :], zeros_f16[:])
                        l, d = 0, 1
                    if h_ > H:
                        nc.sync.dma_start(ut[base:base + F, HB + 1, :],
                                          zeros_f16[:])
                        h_ = H
                    nc.sync.dma_start(ut[base:base + F, d:d + h_ - l, :],
                                      uv[:, l:h_, :])
            uts[k] = (uth, utl)

        # tap col windows: index by "out col reads src col + (idx-1)"
        # idx 0: out [1,256) <- src [0,255); idx 1: full; idx 2: out [0,255) <- src [1,256)
        WIN = [(1, 0, 255), (0, 0, 256), (0, 1, 255)]

        def conv_t_band(k):
            uth, utl = uts[k]
            for g in range(HB // 8):        # 4 groups of 4 chunks per half
                pss = [psA.tile([128, 512], F32, name=f"psA{k}_{g}_{i}",
                                tag=f"psA{i}") for i in range(2)]
                nmm = len(ct_terms) * 9
                imm = 0
                for (ulo, wlo) in ct_terms:
                    ut = utl if ulo else uth
                    wt = wzl if wlo else wzh
                    for b in [1, 0, 2]:
                        # out col c reads u col c+1-b -> window idx (2-b)
                        ob, sb_, width = WIN[2 - b]
                        for a in range(3):
                            tap = a * 3 + b
                            imm += 1
                            for half in range(2):
                                base = 64 * half
                                for cc in range(4):
                                    y = (g * 4 + cc) * 2
                                    col = 32 * cc
                                    rhs = ut[base:base + F,
                                             y + 2 - a:y + 4 - a,
                                             sb_:sb_ + width]
                                    out = pss[half][col:col + 1, :].rearrange(
                                        "p (r c) -> p r c", r=2)[:, :, ob:ob + width]
                                    nc.tensor.matmul(
                                        out, wt[base:base + F, tap:tap + 1], rhs,
                                        start=(imm == 1), stop=(imm == nmm),
                                        tile_position=(base, col),
                                        skip_group_check=True,
                                    )
                for half in range(2):
                    st = stpool.tile([128, 512], F32, name=f"st{k}_{g}_{half}",
                                     tag="st")
                    # psum -> sbuf staging on the scalar engine (ACT is idle)
                    if half == 0:
                        nc.scalar.copy(st[:], pss[half][:])
                    else:
                        nc.vector.tensor_copy(st[:], pss[half][:])
                    gbase = k * 32 + half * 16 + g * 4
                    nc.sync.dma_start(z_sb[gbase:gbase + 4, :], st[0:97:32, :])

        def dx_band(k):
            sl = slice(32 * k, 32 * k + 32)
            x_ = x_sb[sl, :]
            # elementwise chain on gpsimd (SBUF only), conversions on DVE
            nc.gpsimd.tensor_mul(t_sb[sl, :], x_, diag_sb[sl, :])
            nc.vector.scalar_tensor_tensor(
                t_sb[sl, :], t_sb[sl, :], -tau, x_, OP.mult, OP.add)
            nc.gpsimd.tensor_sub(t_sb[sl, :], t_sb[sl, :], z_sb[sl, :])
            nc.vector.scalar_tensor_tensor(
                t_sb[sl, :], bias_sb[sl, :], tau, t_sb[sl, :], OP.mult, OP.add)
            nc.gpsimd.tensor_scalar(
                o0_sb[sl, :], t_sb[sl, :], 0.0, 1.0, OP.max, OP.min)
            # q = 2*out0 - x
            nc.vector.scalar_tensor_tensor(
                q_sb[sl, :], o0_sb[sl, :], 2.0, x_, OP.mult, OP.subtract)
            nc.vector.tensor_copy(qh_sb[sl, :], q_sb[sl, :])
            if CQ_HAS_QLO:
                nc.vector.tensor_sub(t_sb[sl, :], q_sb[sl, :], qh_sb[sl, :])
                nc.vector.tensor_copy(ql_sb[sl, :], t_sb[sl, :])

        def build_q3(k, qplane, q3t):
            """q3t partition (base + i) tile row t = q row (hs - 2 + i + t),
            zeros outside the image."""
            r0 = k * R
            for half in range(2):
                base = 64 * half
                hs = r0 + half * HB
                for i in range(3):
                    p = base + i
                    rlo = hs - 2 + i
                    rhi = rlo + HB + 2
                    dlo = 0
                    if rlo < 0:
                        for d in range(-rlo):
                            nc.sync.dma_start(q3t[p:p + 1, d, :],
                                              zeros_f16[0:1, :])
                        dlo, rlo = -rlo, 0
                    if rhi > H:
                        for d in range(rhi - H):
                            nc.sync.dma_start(
                                q3t[p:p + 1, HB + 2 - (rhi - H) + d, :],
                                zeros_f16[0:1, :])
                        rhi = H
                    nrows = rhi - rlo
                    plo = rlo // 2
                    if rlo % 2 == 1:
                        nc.sync.dma_start(q3t[p:p + 1, dlo, :],
                                          qplane[plo:plo + 1, 256:512])
                        plo += 1
                        dlo += 1
                        nrows -= 1
                    neven = nrows // 2
                    if neven:
                        nc.sync.dma_start(
                            q3t[p:p + 1, dlo:dlo + 2 * neven, :],
                            qplane[plo:plo + neven, :])
                    if nrows % 2 == 1:
                        nc.sync.dma_start(
                            q3t[p:p + 1, dlo + 2 * neven, :],
                            qplane[plo + neven:plo + neven + 1, 0:256])

        def conv_q_band(k):
            q3h = q3pool.tile([128, HB + 2, W], F16, name=f"q3h{k}", tag="q3h")
            build_q3(k, qh_sb, q3h)
            q3l = None
            if CQ_HAS_QLO:
                q3l = q3pool.tile([128, HB + 2, W], F16, name=f"q3l{k}",
                                  tag="q3l")
                build_q3(k, ql_sb, q3l)
            uth, utl = uts[k]
            r0 = k * R
            for pair in range(HB // 2):     # 16 chunk-pairs per band
                ps = psQ.tile([128, 512], F32, name=f"psQ{k}_{pair}", tag="psQ")
                y = pair * 2
                for half in range(2):
                    base = 64 * half
                    # alternate col sets by pair parity so two consecutive
                    # pairs' matmul streams use disjoint (row, col) groups
                    # and run concurrently on the PE array
                    cb = 64 * (half ^ (pair & 1))
                    # u identity-accumulate (exact fp16 hi+lo pass-through)
                    nc.tensor.matmul(
                        ps[cb:cb + F, :], eye[base:base + F, :],
                        uth[base:base + F, y + 1:y + 3, :],
                        start=True, stop=False, tile_position=(base, cb),
                        skip_group_check=True)
                    nc.tensor.matmul(
                        ps[cb:cb + F, :], eye[base:base + F, :],
                        utl[base:base + F, y + 1:y + 3, :],
                        start=False, stop=False, tile_position=(base, cb),
                        skip_group_check=True)
                    nmm = 3 * len(cq_terms)
                    imm = 0
                    for (qlo, wlo) in cq_terms:
                        q3 = q3l if qlo else q3h
                        wt = wq3l if wlo else wq3
                        for j in [1, 0, 2]:
                            ob, sb_, width = WIN[j]
                            imm += 1
                            rhs = q3[base:base + 3, y + 1:y + 3,
                                     sb_:sb_ + width]
                            out = ps[cb:cb + F, :].rearrange(
                                "p (r c) -> p r c", r=2)[:, :, ob:ob + width]
                            nc.tensor.matmul(
                                out, wt[base:base + 3, j, :], rhs,
                                start=False, stop=(imm == nmm),
                                tile_position=(base, cb),
                                skip_group_check=True)
                # fused epilogue: out1 = clip(psum, -lambd, lambd)
                o1 = o1pool.tile([128, 512], F32, name=f"o1_{k}_{pair}",
                                 tag="o1")
                nc.vector.tensor_scalar(
                    o1[:], ps[:], lambd_, -lambd_, OP.min, OP.max)
                # single DMA: partitions (colset, ch) -> rows; on odd pairs
                # the halves sit swapped in psum partitions
                dst = out1_d.ap().rearrange(
                    "f (bl h r c) -> f bl h r c",
                    bl=NBANDS, h=2, c=W)[:, k, :, y:y + 2, :]
                dst = dst.transpose([1, 0, 2, 3])
                if pair & 1:
                    nc.sync.dma_start(dst[1], o1[0:64, :])
                    nc.sync.dma_start(dst[0], o1[64:128, :])
                else:
                    nc.sync.dma_start(dst, o1[:])

        # ---------------- schedule
        for k in range(NBANDS):
            load_u(k)
            conv_t_band(k)
            dx_band(k)
            if k >= 1:
                conv_q_band(k - 1)
        conv_q_band(NBANDS - 1)
        nc.sync.dma_start(out0_d.ap(), o0_sb[:])
        if debug:
            nc.sync.dma_start(dbgz_d.ap(), z_sb[:])
            nc.sync.dma_start(dbgq_d.ap(), q_sb[:])

    nc.compile()
    return nc


# ---------------------------------------------------------------- wrapper

def kernel(x_in, u_in, bias, beta, lambd, HtH_diag, w, log_sigma):
    x_in = np.asarray(x_in, np.float32)
    u_in = np.asarray(u_in, np.float32)
    bias = np.asarray(bias, np.float32)
    HtH_diag = np.asarray(HtH_diag, np.float32)
    w = np.asarray(w, np.float32)
    beta_f = float(np.asarray(beta).reshape(-1)[0])
    lambd_f = float(np.asarray(lambd).reshape(-1)[0])
    log_sigma_f = float(np.asarray(log_sigma).reshape(-1)[0])

    lip2 = _power_iteration_lip2(w)
    sigma = float(np.exp(np.float32(log_sigma_f)) + np.float32(0.05))
    tau = float(np.float32(0.99) / np.float32(beta_f / 2 + sigma * lip2))

    # pick precision terms from the actual quantization-error scales.
    # conv_t folds tau into the weights, and tau ~ 1/lip2 ~ 1/std(w)^2, so
    # std(tau*w) self-normalizes: plain fp16 suffices unless it is large
    # (measured: std(tau*w)=0.0124 -> ~4.5e-4 out0 err with plain fp16).
    ct_std = float(np.std(np.float32(tau) * w))
    ct_terms = [(0, 0), (0, 1)] if ct_std > 0.02 else [(0, 0)]
    # conv_q error ~ 3 taps * std(sigma*w) * 2^-11 * |q|; escalate to the
    # full 3-term product when the plain estimate threatens ~1e-3 absolute.
    w_std = float(np.std(np.float32(sigma) * w))
    cq_err_est = 3.0 * w_std * 2.0 ** -11 * 3.0
    if cq_err_est > 6e-4:
        cq_terms = [(0, 0), (1, 0), (0, 1)]
    else:
        cq_terms = [(0, 0), (0, 1)]

    nc = _build_program(
        tau, sigma, lambd_f, ct_terms, cq_terms,
        debug=bool(int(os.environ.get("KERNEL_DEBUG", "0"))))
    _LAST_PROGRAM[0] = nc

    w9 = w.reshape(F, 9)
    wzh, wzl = _f16_split(np.float32(tau) * w9)
    # wq[i, j*F + f] = sigma * w[f, 0, i, j]
    wq = np.ascontiguousarray(
        (np.float32(sigma) * w.reshape(F, 3, 3)).transpose(1, 2, 0)
    ).reshape(3, 3 * F)
    wqh, wql = _f16_split(wq)
    eye = np.eye(F, dtype=np.float16)
    diag_plane = np.ascontiguousarray(HtH_diag.reshape(128, 512))

    in_maps = []
    for b in range(B):
        uh, ul = _f16_split(u_in[b].reshape(F, H * W))
        m = {
            "uh": uh, "ul": ul,
            "x": np.ascontiguousarray(x_in[b, 0].reshape(128, 512)),
            "bias": np.ascontiguousarray(bias[b].reshape(128, 512)),
            "diag": diag_plane,
            "wzh": wzh, "wq": wqh, "eye": eye,
        }
        if any(t[1] for t in ct_terms):
            m["wzl"] = wzl
        if any(t[1] for t in cq_terms):
            m["wql"] = wql
        in_maps.append(m)

    res = bass_utils.run_bass_kernel_spmd(
        nc, in_maps, core_ids=list(range(B)), trace=_TRACE[0])
    _LAST_RESULTS[0] = res

    out0 = np.stack([res.results[b]["out0"].reshape(1, H, W)
                     for b in range(B)])
    out1 = np.stack([res.results[b]["out1"].reshape(F, H, W)
                     for b in range(B)])
    return out0, out1
